# revision 35
# baseline (speedup 1.0000x reference)
"""DropBlock kernel for Trainium2, 8 NeuronCores, batch-sharded data parallel.

Reference computation (B,C,H,W = 128,64,56,56, block=5, gamma=0.02):
    drop    = (noise < gamma)                       # (B,C,52,52) corner drops
    dilated = maxpool5x5_full_pad(drop)             # (B,C,56,56)
    block_mask = 1 - dilated
    out = block_mask * x * (numel / sum(block_mask))

Kernel formulation (exact -- all intermediates are small integers):
    m = (noise < gamma) in {0,1}                    # Pool tensor_scalar
    C = conv5x5_fullpad(m)   (separable: vertical then horizontal box sum)
    block_mask = (C < 0.5)   == (C == 0) == not dilated
    count = sum(block_mask) via fused ACT accum; AllReduce across 8 cores.

Engine assignment (each phase-1 stage on its own engine, pipelined by
tile; the Pool engine only supports TensorScalar/memset/ISA ops and the
PE only contracts over partitions -- both constraints shaped this):
    Pool: m = tensor_scalar(noise, is_lt gamma)  (the only legal Pool op)
    PE  : vertical 5-row box sum as 5 identity-matmuls with row-shifted
          moving APs accumulated into PSUM (416-col row-aligned chunks;
          contraction over partitions is a no-op via the identity)
    ACT : PSUM -> SBUF bf16 drain into the 60-wide zero-padded layout,
          and the final mask write (fp8) + fused count accumulation
    DVE : horizontal box sum (3 log-step adds), threshold (tensor_scalar
          is_lt at 4x), and the phase-2 (x*scale)*mask multiplies
Tile 7's horizontal chain runs in row-thirds to shorten the critical
path into the count -> AllReduce -> scale; stores start right after.
"""

import sys

sys.path.insert(0, "/opt/trn_rl_repo")

import numpy as np

import concourse.bacc as bacc
import concourse.bass as bass
import concourse.tile as tile
import concourse.mybir as mybir
from concourse.masks import make_identity
from concourse.bass_utils import run_bass_kernel_spmd

N_CORES = 8
B, C, H, W = 128, 64, 56, 56
BLK = 5
GAMMA = 0.02
NH, NW = H - (BLK - 1), W - (BLK - 1)  # 52, 52 noise dims
B_SH = B // N_CORES  # 16 batches per core
IMGS = B_SH * C  # 1024 images per core
P = 128  # partitions
NTILES = IMGS // P  # 8 tiles per core
NPIX = NH * NW  # 2704 noise pixels/image
OPIX = H * W  # 3136 out pixels/image
TROWS = NH + 2 * (BLK - 1)  # 60 rows in zero-padded indicator buffer
TFLAT = TROWS * NW  # 3120
VPW = NW + 2 * (BLK - 1)  # 60 cols in zero-padded horizontal buffer
HV = H * VPW  # 3360
COUNT_M = float(B * C * H * W)  # 25690112.0

F32 = mybir.dt.float32
BF16 = mybir.dt.bfloat16
FP8 = mybir.dt.float8e4
MULT = mybir.AluOpType.mult
ADD = mybir.AluOpType.add
IS_LT = mybir.AluOpType.is_lt
IDENT = mybir.ActivationFunctionType.Identity

CHROWS = 8  # V rows per PE/PSUM chunk (416 cols = 1 PSUM bank)
NCHUNK = H // CHROWS  # 7 chunks per tile
CHW = CHROWS * NW  # 416

_CACHE = {}


def _build(single_core=False, repeat=1, no_cc=False):
    """Build + compile the SPMD bass module once.

    single_core=True builds a collective-free variant (per-core count used
    as the global count) for cost-model simulation only.  repeat>1 unrolls
    the pipeline k times (benchmarking only).  no_cc=True skips the
    AllReduce on the 8-core build (timing probe only).
    """
    nc = bacc.Bacc("TRN2", target_bir_lowering=False, debug=False,
                   num_devices=1 if single_core else N_CORES)
    noise_ap = nc.dram_tensor("noise", [IMGS, NPIX], F32,
                              kind="ExternalInput").ap()
    x_ap = nc.dram_tensor("x", [IMGS, OPIX], F32, kind="ExternalInput").ap()
    out_ap = nc.dram_tensor("out", [IMGS, OPIX], F32,
                            kind="ExternalOutput").ap()

    with tile.TileContext(nc) as tc:
        with (
            tc.tile_pool(name="nraw", bufs=3) as nraw_pool,
            tc.tile_pool(name="work", bufs=1) as work_pool,
            tc.tile_pool(name="vp", bufs=2) as vp_pool,
            tc.tile_pool(name="dmin", bufs=2) as dmin_pool,
            tc.tile_pool(name="mask", bufs=1) as mask_pool,
            tc.tile_pool(name="stats", bufs=1) as stats_pool,
            tc.tile_pool(name="xio", bufs=8) as x_pool,
            tc.tile_pool(name="psum", bufs=8, space="PSUM") as psum_pool,
            tc.tile_pool(name="dram", bufs=1, space="DRAM") as dram_pool,
        ):
            mask_store = mask_pool.tile([P, NTILES * OPIX], FP8)
            # count columns: tiles 0..6 -> 0..6, tile-7 thirds -> 7..9
            partials = stats_pool.tile([P, 10], F32)
            ident = stats_pool.tile([P, P], BF16)
            make_identity(nc, ident[:])
            # warm the ACT Identity table during the DMA lead-in
            warm = stats_pool.tile([P, 1], F32)
            warm2 = stats_pool.tile([P, 1], F32)
            nc.scalar.activation(warm[:], warm[:], IDENT,
                                 accum_out=warm2[:, 0:1])

            pools = dict(nraw=nraw_pool, work=work_pool, vp=vp_pool,
                         dmin=dmin_pool, x=x_pool, psum=psum_pool,
                         stats=stats_pool, dram=dram_pool)
            for rep in range(repeat):
                _emit_once(nc, tc, noise_ap, x_ap, out_ap, mask_store,
                           partials, ident, pools, single_core or no_cc,
                           rep)

    nc.compile()
    return nc


def _emit_once(nc, tc, noise_ap, x_ap, out_ap, mask_store, partials, ident,
               pools, single_core, rep):
    nraw_pool = pools["nraw"]
    work_pool = pools["work"]
    vp_pool = pools["vp"]
    dmin_pool = pools["dmin"]
    x_pool = pools["x"]
    stats_pool = pools["stats"]
    dram_pool = pools["dram"]
    psum_pool = pools["psum"]

    xts = {}
    state = {}
    pre = {}
    # pre-allocate the pad-carrying buffers and memset the ZERO pads once,
    # up-front (in-loop memsets would queue behind real Pool/DVE work)
    pre["mb"] = [work_pool.tile([P, TFLAT], BF16, name=f"mbp{rep}_{i}",
                                tag="mb", bufs=2) for i in range(2)]
    pre["vp"] = [vp_pool.tile([P, HV], BF16, name=f"vpp{rep}_{i}",
                              tag="vp") for i in range(2)]
    if rep == 0:
        for mbp in pre["mb"]:
            nc.gpsimd.memset(mbp[:, 0:(BLK - 1) * NW], 0.0)
            nc.gpsimd.memset(mbp[:, (NH + BLK - 1) * NW:TFLAT], 0.0)
        for vpp in pre["vp"]:
            v3 = vpp[:].rearrange("p (h w) -> p h w", w=VPW)
            nc.gpsimd.memset(v3[:, :, 0:BLK - 1], 0.0)
            nc.gpsimd.memset(v3[:, :, W:VPW], 0.0)

    def emit_front(t):
        """noise loads, x prefetch, Pool indicator, PE conv, ACT drain."""
        nhs = []
        for hf in range(2):
            nh = nraw_pool.tile([P, NPIX // 2], F32,
                                name=f"nh{rep}_{t}_{hf}", tag="nh")
            nhs.append(nh)
            base = hf * (NPIX // 2)
            nc.sync.dma_start(nh[:],
                              noise_ap[bass.ts(t, P),
                                       base:base + NPIX // 2])
        # x loads ride the other HWDGE queue (ACT's), in halves
        xts[t] = x_pool.tile([P, OPIX], F32, name=f"xt{rep}_{t}", tag="xt")
        with tc.tile_wait_until(t * 10.0 + 5.0):
            nc.scalar.dma_start(xts[t][:, 0:OPIX // 2],
                                x_ap[bass.ts(t, P), 0:OPIX // 2])
            nc.scalar.dma_start(xts[t][:, OPIX // 2:OPIX],
                                x_ap[bass.ts(t, P), OPIX // 2:OPIX])

        # drop indicator m in {0,1}, rows 4..55 of the 60-row zero-padded
        # buffer; exact f32 compare against gamma, bf16 out (Pool engine)
        mb = pre["mb"][t] if t < 2 else work_pool.tile(
            [P, TFLAT], BF16, name=f"mb{rep}_{t}", tag="mb", bufs=2)
        for q in range(2):
            r0 = (BLK - 1) + q * (NH // 2)
            nc.gpsimd.tensor_scalar(
                mb[:, r0 * NW:(r0 + NH // 2) * NW],
                nhs[q][:], GAMMA, None, IS_LT)

        # vertical 5-row box sum on PE: V[i] = sum_j m[i + 52j], computed
        # as 5 identity-matmuls with row-shifted moving APs accumulated in
        # PSUM, in 8-row chunks (416 cols = 1 bank); ACT drains each chunk
        # into the padded horizontal buffer as bf16 (values 0..5, exact)
        vp = pre["vp"][t] if t < 2 else vp_pool.tile(
            [P, HV], BF16, name=f"vp{rep}_{t}", tag="vp")
        vp3 = vp[:].rearrange("p (h w) -> p h w", w=VPW)
        for c in range(NCHUNK):
            pt = psum_pool.tile([P, CHW], F32, name=f"ps{rep}_{t}_{c}",
                                tag="ps")
            for j in range(BLK):
                nc.tensor.matmul(
                    pt[:], ident[:],
                    mb[:, c * CHW + NW * j:c * CHW + NW * j + CHW],
                    start=(j == 0), stop=(j == BLK - 1))
            pt3 = pt[:].rearrange("p (h w) -> p h w", w=NW)
            dst = vp3[:, c * CHROWS:(c + 1) * CHROWS, BLK - 1:BLK - 1 + NW]
            nc.scalar.activation(dst, pt3[:, :, :], IDENT)
        state[t] = {"vp": vp}
        return vp

    def emit_h(t, rows=None):
        """horizontal box sum + threshold on DVE for row range [r0, r1)."""
        r0, r1 = (0, H) if rows is None else rows
        vp = state[t]["vp"]
        a2 = state[t].get("a2")
        if a2 is None:
            a2 = work_pool.tile([P, HV], BF16, name=f"a2{rep}_{t}",
                                tag="a2", bufs=2)
            state[t]["a2"] = a2
            b2 = work_pool.tile([P, HV], BF16, name=f"b2{rep}_{t}",
                                tag="b2", bufs=1)
            state[t]["b2"] = b2
            cs = dmin_pool.tile([P, OPIX], BF16, name=f"cs{rep}_{t}",
                                tag="cs")
            state[t]["cs"] = cs
        b2 = state[t]["b2"]
        cs = state[t]["cs"]
        alo, ahi = r0 * VPW, min((r1 - 1) * VPW + W + 2, HV - 1)
        nc.vector.tensor_tensor(a2[:, alo:ahi], vp[:, alo:ahi],
                                vp[:, alo + 1:ahi + 1], ADD)
        blo, bhi = r0 * VPW, min((r1 - 1) * VPW + W, HV - 2)
        nc.vector.tensor_tensor(b2[:, blo:bhi], a2[:, blo:bhi],
                                a2[:, blo + 2:bhi + 2], ADD)
        b23 = b2[:].rearrange("p (h w) -> p h w", w=VPW)
        vp3 = vp[:].rearrange("p (h w) -> p h w", w=VPW)
        cs3 = cs[:].rearrange("p (h w) -> p h w", w=W)
        nc.vector.tensor_tensor(cs3[:, r0:r1, :], b23[:, r0:r1, 0:W],
                                vp3[:, r0:r1, BLK - 1:VPW], ADD)
        # block_mask = (C == 0): threshold in-place at 4x
        nc.vector.tensor_scalar(cs3[:, r0:r1, :], cs3[:, r0:r1, :], 0.5,
                                None, IS_LT)
        return cs

    def emit_mask(t, col, rows=None):
        """mask -> fp8 store + fused count accum on ACT."""
        r0, r1 = (0, H) if rows is None else rows
        cs = state[t]["cs"]
        msl = mask_store[:].rearrange("p (t h w) -> p t h w", t=NTILES, w=W)
        cs3 = cs[:].rearrange("p (h w) -> p h w", w=W)
        nc.scalar.activation(msl[:, t, r0:r1, :], cs3[:, r0:r1, :], IDENT,
                             accum_out=partials[:, col:col + 1])

    RB = 10.0  # pseudo-ms per tile: scheduler ordering only
    for t in range(NTILES):
        with tc.tile_wait_until(t * RB):
            emit_front(t)
        if t >= 1 and t - 1 < NTILES - 1:
            with tc.tile_wait_until(t * RB + 2.5):
                emit_h(t - 1)
            with tc.tile_wait_until(t * RB + 3.0):
                emit_mask(t - 1, t - 1)

    # tile 7 in row-thirds to shorten the critical path into the count
    R3 = [(0, 19), (19, 38), (38, H)]
    with tc.tile_wait_until(8 * RB):
        phead = None
        for i, (r0, r1) in enumerate(R3):
            emit_h(7, rows=(r0, r1))
            emit_mask(7, 7 + i, rows=(r0, r1))
            if i == 1:
                phead = stats_pool.tile([P, 1], F32, name=f"phead{rep}",
                                        tag="phead")
                nc.vector.tensor_reduce(phead[:], partials[:, 0:9],
                                        mybir.AxisListType.X, ADD)
        ptot = stats_pool.tile([P, 1], F32, name=f"ptot{rep}", tag="ptot")
        nc.vector.tensor_tensor(ptot[:], phead[:], partials[:, 9:10], ADD)

        # ---------- global count -> scale = M / count_ones ----------
        from concourse import bass_isa
        pall = stats_pool.tile([P, 1], F32, name=f"pall{rep}", tag="pall")
        nc.gpsimd.partition_all_reduce(pall[:], ptot[:], channels=P,
                                       reduce_op=bass_isa.ReduceOp.add)
        if single_core:
            tot_sb = pall
        else:
            cc_in = dram_pool.tile([P, 1], F32, name=f"cc_in{rep}",
                                   tag="cc_in")
            cc_out = dram_pool.tile([P, 1], F32, name=f"cc_out{rep}",
                                    tag="cc_out")
            nc.scalar.dma_start(cc_in[:], pall[:])
            nc.gpsimd.collective_compute(
                "AllReduce", ADD,
                replica_groups=[list(range(N_CORES))],
                ins=[cc_in.opt()], outs=[cc_out.opt()])
            tot_sb = stats_pool.tile([P, 1], F32, name=f"tot{rep}",
                                     tag="tot")
            nc.scalar.dma_start(tot_sb[:], cc_out[:])
        # scale = M / count: fold 1/M into the reciprocal input
        rin = stats_pool.tile([P, 1], F32, name=f"rin{rep}", tag="rin")
        nc.vector.tensor_scalar_mul(rin[:], tot_sb[:], 1.0 / COUNT_M)
        scale_sb = stats_pool.tile([P, 1], F32, name=f"scale{rep}",
                                   tag="scale")
        nc.vector.reciprocal(scale_sb[:], rin[:])

    # ---------------- phase 2: out = (x*scale)*mask ----------------
    # halves: the first store launches after half a multiply; stores
    # alternate the two HWDGE queues
    HX = OPIX // 2
    with tc.tile_wait_until(9 * RB):
        for t in range(NTILES):
            xt = xts[t]
            qeng = nc.sync if t % 2 == 0 else nc.scalar
            for h in range(2):
                sl = slice(h * HX, (h + 1) * HX)
                nc.vector.scalar_tensor_tensor(
                    xt[:, sl], xt[:, sl], scale_sb[:, 0:1],
                    mask_store[:,
                               t * OPIX + h * HX:t * OPIX + (h + 1) * HX],
                    MULT, MULT)
                qeng.dma_start(out_ap[bass.ts(t, P), sl], xt[:, sl])


def _get_nc():
    if "nc" not in _CACHE:
        _CACHE["nc"] = _build()
    return _CACHE["nc"]


def kernel(x: np.ndarray, noise: np.ndarray) -> np.ndarray:
    x = np.asarray(x, dtype=np.float32)
    noise = np.asarray(noise, dtype=np.float32)
    assert x.shape == (B, C, H, W) and noise.shape == (B, C, NH, NW)
    nc = _get_nc()
    in_maps = []
    for i in range(N_CORES):
        xs = np.ascontiguousarray(x[i * B_SH:(i + 1) * B_SH]).reshape(
            IMGS, OPIX)
        ns = np.ascontiguousarray(noise[i * B_SH:(i + 1) * B_SH]).reshape(
            IMGS, NPIX)
        in_maps.append({"x": xs, "noise": ns})
    res = run_bass_kernel_spmd(nc, in_maps, list(range(N_CORES)))
    out = np.empty((B, C, H, W), dtype=np.float32)
    for i in range(N_CORES):
        out[i * B_SH:(i + 1) * B_SH] = res.results[i]["out"].reshape(
            B_SH, C, H, W)
    return out


# revision 40
# speedup vs baseline: 1.0362x; 1.0362x over previous
"""DropBlock kernel for Trainium2, 8 NeuronCores, batch-sharded data parallel.

Reference computation (B,C,H,W = 128,64,56,56, block=5, gamma=0.02):
    drop    = (noise < gamma)                       # (B,C,52,52) corner drops
    dilated = maxpool5x5_full_pad(drop)             # (B,C,56,56)
    block_mask = 1 - dilated
    out = block_mask * x * (numel / sum(block_mask))

Kernel formulation (exact -- all intermediates are small integers):
    m = (noise < gamma) in {0,1}                    # Pool tensor_scalar
    C = conv5x5_fullpad(m)   (separable: vertical then horizontal box sum)
    block_mask = (C < 0.5)   == (C == 0) == not dilated
    count = sum(block_mask) via fused ACT accum; AllReduce across 8 cores.

Engine assignment (each phase-1 stage on its own engine, pipelined by
tile; the Pool engine only supports TensorScalar/memset/ISA ops and the
PE only contracts over partitions -- both constraints shaped this):
    Pool: m = tensor_scalar(noise, is_lt gamma)  (the only legal Pool op)
    PE  : vertical 5-row box sum as 5 identity-matmuls with row-shifted
          moving APs accumulated into PSUM (416-col row-aligned chunks;
          contraction over partitions is a no-op via the identity)
    ACT : PSUM -> SBUF bf16 drain into the 60-wide zero-padded layout,
          and the final mask write (fp8) + fused count accumulation
    DVE : horizontal box sum (3 log-step adds), threshold (tensor_scalar
          is_lt at 4x), and the phase-2 (x*scale)*mask multiplies
Tile 7's horizontal chain runs in row-thirds to shorten the critical
path into the count -> AllReduce -> scale; stores start right after.
"""

import sys

sys.path.insert(0, "/opt/trn_rl_repo")

import numpy as np

import concourse.bacc as bacc
import concourse.bass as bass
import concourse.tile as tile
import concourse.mybir as mybir
from concourse.masks import make_identity
from concourse.bass_utils import run_bass_kernel_spmd

N_CORES = 8
B, C, H, W = 128, 64, 56, 56
BLK = 5
GAMMA = 0.02
NH, NW = H - (BLK - 1), W - (BLK - 1)  # 52, 52 noise dims
B_SH = B // N_CORES  # 16 batches per core
IMGS = B_SH * C  # 1024 images per core
P = 128  # partitions
NTILES = IMGS // P  # 8 tiles per core
NPIX = NH * NW  # 2704 noise pixels/image
OPIX = H * W  # 3136 out pixels/image
TROWS = NH + 2 * (BLK - 1)  # 60 rows in zero-padded indicator buffer
TFLAT = TROWS * NW  # 3120
VPW = NW + 2 * (BLK - 1)  # 60 cols in zero-padded horizontal buffer
HV = H * VPW  # 3360
COUNT_M = float(B * C * H * W)  # 25690112.0

F32 = mybir.dt.float32
BF16 = mybir.dt.bfloat16
FP8 = mybir.dt.float8e4
MULT = mybir.AluOpType.mult
ADD = mybir.AluOpType.add
IS_LT = mybir.AluOpType.is_lt
IDENT = mybir.ActivationFunctionType.Identity

CHROWS = 8  # V rows per PE/PSUM chunk (416 cols = 1 PSUM bank)
NCHUNK = H // CHROWS  # 7 chunks per tile
CHW = CHROWS * NW  # 416

_CACHE = {}


def _build(single_core=False, repeat=1, no_cc=False):
    """Build + compile the SPMD bass module once.

    single_core=True builds a collective-free variant (per-core count used
    as the global count) for cost-model simulation only.  repeat>1 unrolls
    the pipeline k times (benchmarking only).  no_cc=True skips the
    AllReduce on the 8-core build (timing probe only).
    """
    nc = bacc.Bacc("TRN2", target_bir_lowering=False, debug=False,
                   num_devices=1 if single_core else N_CORES)
    noise_ap = nc.dram_tensor("noise", [IMGS, NPIX], F32,
                              kind="ExternalInput").ap()
    x_ap = nc.dram_tensor("x", [IMGS, OPIX], F32, kind="ExternalInput").ap()
    out_ap = nc.dram_tensor("out", [IMGS, OPIX], F32,
                            kind="ExternalOutput").ap()

    with tile.TileContext(nc) as tc:
        with (
            tc.tile_pool(name="nraw", bufs=4) as nraw_pool,
            tc.tile_pool(name="work", bufs=1) as work_pool,
            tc.tile_pool(name="vp", bufs=2) as vp_pool,
            tc.tile_pool(name="dmin", bufs=2) as dmin_pool,
            tc.tile_pool(name="mask", bufs=1) as mask_pool,
            tc.tile_pool(name="stats", bufs=1) as stats_pool,
            tc.tile_pool(name="xio", bufs=8) as x_pool,
            tc.tile_pool(name="psum", bufs=8, space="PSUM") as psum_pool,
            tc.tile_pool(name="dram", bufs=1, space="DRAM") as dram_pool,
        ):
            mask_store = mask_pool.tile([P, NTILES * OPIX], FP8)
            # count columns: tiles 0..6 -> 0..6, tile-7 thirds -> 7..9
            partials = stats_pool.tile([P, 10], F32)
            ident = stats_pool.tile([P, P], BF16)
            make_identity(nc, ident[:])
            # warm the ACT Identity table during the DMA lead-in
            warm = stats_pool.tile([P, 1], F32)
            warm2 = stats_pool.tile([P, 1], F32)
            nc.scalar.activation(warm[:], warm[:], IDENT,
                                 accum_out=warm2[:, 0:1])
            # warm the PE p-state: ~3us of dummy matmuls during the DMA
            # lead-in so real conv matmuls start at full clock
            wps = psum_pool.tile([P, P], F32, name="warmps", tag="warmps",
                                 bufs=1)
            for _ in range(18):
                nc.tensor.matmul(wps[:], ident[:], ident[:], start=True,
                                 stop=True)

            pools = dict(nraw=nraw_pool, work=work_pool, vp=vp_pool,
                         dmin=dmin_pool, x=x_pool, psum=psum_pool,
                         stats=stats_pool, dram=dram_pool)
            for rep in range(repeat):
                _emit_once(nc, tc, noise_ap, x_ap, out_ap, mask_store,
                           partials, ident, pools, single_core or no_cc,
                           rep)

    nc.compile()
    return nc


def _emit_once(nc, tc, noise_ap, x_ap, out_ap, mask_store, partials, ident,
               pools, single_core, rep):
    nraw_pool = pools["nraw"]
    work_pool = pools["work"]
    vp_pool = pools["vp"]
    dmin_pool = pools["dmin"]
    x_pool = pools["x"]
    stats_pool = pools["stats"]
    dram_pool = pools["dram"]
    psum_pool = pools["psum"]

    xts = {}
    state = {}
    pre = {}
    # pre-allocate the pad-carrying buffers and memset the ZERO pads once,
    # up-front (in-loop memsets would queue behind real Pool/DVE work)
    pre["mb"] = [work_pool.tile([P, TFLAT], BF16, name=f"mbp{rep}_{i}",
                                tag="mb", bufs=3) for i in range(3)]
    pre["vp"] = [vp_pool.tile([P, HV], BF16, name=f"vpp{rep}_{i}",
                              tag="vp") for i in range(2)]
    if rep == 0:
        for mbp in pre["mb"]:
            nc.gpsimd.memset(mbp[:, 0:(BLK - 1) * NW], 0.0)
            nc.gpsimd.memset(mbp[:, (NH + BLK - 1) * NW:TFLAT], 0.0)
        for vpp in pre["vp"]:
            v3 = vpp[:].rearrange("p (h w) -> p h w", w=VPW)
            nc.gpsimd.memset(v3[:, :, 0:BLK - 1], 0.0)
            nc.gpsimd.memset(v3[:, :, W:VPW], 0.0)

    def emit_front(t):
        """noise loads, x prefetch, Pool indicator, PE conv, ACT drain."""
        # two half-tiles in the nh ring; tile 0 issues quarter-DMAs and
        # quarter indicator ops into the same buffers so the pipeline
        # fill chases at 13-row granularity
        nhs = []
        for hf in range(2):
            nh = nraw_pool.tile([P, NPIX // 2], F32,
                                name=f"nh{rep}_{t}_{hf}", tag="nh")
            nhs.append(nh)
            base = hf * (NPIX // 2)
            if t == 0:
                nc.sync.dma_start(nh[:, 0:NPIX // 4],
                                  noise_ap[bass.ts(t, P),
                                           base:base + NPIX // 4])
                nc.sync.dma_start(nh[:, NPIX // 4:NPIX // 2],
                                  noise_ap[bass.ts(t, P),
                                           base + NPIX // 4:base + NPIX // 2])
            else:
                nc.sync.dma_start(nh[:],
                                  noise_ap[bass.ts(t, P),
                                           base:base + NPIX // 2])
        # x loads ride the other HWDGE queue (ACT's), in halves
        xts[t] = x_pool.tile([P, OPIX], F32, name=f"xt{rep}_{t}", tag="xt")
        with tc.tile_wait_until(t * 10.0 + 5.0):
            nc.scalar.dma_start(xts[t][:, 0:OPIX // 2],
                                x_ap[bass.ts(t, P), 0:OPIX // 2])
            nc.scalar.dma_start(xts[t][:, OPIX // 2:OPIX],
                                x_ap[bass.ts(t, P), OPIX // 2:OPIX])

        # drop indicator m in {0,1}, rows 4..55 of the 60-row zero-padded
        # buffer; exact f32 compare against gamma, bf16 out (Pool engine)
        mb = pre["mb"][t] if t < 3 else work_pool.tile(
            [P, TFLAT], BF16, name=f"mb{rep}_{t}", tag="mb", bufs=3)
        nparts = 4 if t == 0 else 2
        rows_per = NH // nparts  # 13 or 26 noise rows per part
        for q in range(nparts):
            r0 = (BLK - 1) + q * rows_per
            nh = nhs[q // 2] if nparts == 4 else nhs[q]
            lo = (q % 2) * (NPIX // 4) if nparts == 4 else 0
            nc.gpsimd.tensor_scalar(
                mb[:, r0 * NW:(r0 + rows_per) * NW],
                nh[:, lo:lo + rows_per * NW], GAMMA, None, IS_LT)

        # vertical 5-row box sum on PE: V[i] = sum_j m[i + 52j], computed
        # as 5 identity-matmuls with row-shifted moving APs accumulated in
        # PSUM, in 8-row chunks (416 cols = 1 bank); ACT drains each chunk
        # into the padded horizontal buffer as bf16 (values 0..5, exact)
        vp = pre["vp"][t] if t < 2 else vp_pool.tile(
            [P, HV], BF16, name=f"vp{rep}_{t}", tag="vp")
        vp3 = vp[:].rearrange("p (h w) -> p h w", w=VPW)
        for c in range(NCHUNK):
            pt = psum_pool.tile([P, CHW], F32, name=f"ps{rep}_{t}_{c}",
                                tag="ps", bufs=7)
            for j in range(BLK):
                nc.tensor.matmul(
                    pt[:], ident[:],
                    mb[:, c * CHW + NW * j:c * CHW + NW * j + CHW],
                    start=(j == 0), stop=(j == BLK - 1))
            pt3 = pt[:].rearrange("p (h w) -> p h w", w=NW)
            dst = vp3[:, c * CHROWS:(c + 1) * CHROWS, BLK - 1:BLK - 1 + NW]
            if c >= 5 and t < NTILES - 1:
                # last two chunks drain on DVE (scheduled after the
                # previous tile's h-chain) to unload the ACT stream;
                # tile 7 keeps ACT drains so its h-thirds start sooner
                with tc.tile_wait_until(t * 10.0 + 2.9):
                    nc.vector.tensor_copy(dst, pt3[:, :, :])
            else:
                nc.scalar.activation(dst, pt3[:, :, :], IDENT)
        state[t] = {"vp": vp}
        return vp

    def emit_h(t, rows=None):
        """horizontal box sum + threshold on DVE for row range [r0, r1)."""
        r0, r1 = (0, H) if rows is None else rows
        vp = state[t]["vp"]
        a2 = state[t].get("a2")
        if a2 is None:
            a2 = work_pool.tile([P, HV], BF16, name=f"a2{rep}_{t}",
                                tag="a2", bufs=2)
            state[t]["a2"] = a2
            b2 = work_pool.tile([P, HV], BF16, name=f"b2{rep}_{t}",
                                tag="b2", bufs=1)
            state[t]["b2"] = b2
            cs = dmin_pool.tile([P, OPIX], BF16, name=f"cs{rep}_{t}",
                                tag="cs")
            state[t]["cs"] = cs
        b2 = state[t]["b2"]
        cs = state[t]["cs"]
        alo, ahi = r0 * VPW, min((r1 - 1) * VPW + W + 2, HV - 1)
        nc.vector.tensor_tensor(a2[:, alo:ahi], vp[:, alo:ahi],
                                vp[:, alo + 1:ahi + 1], ADD)
        blo, bhi = r0 * VPW, min((r1 - 1) * VPW + W, HV - 2)
        nc.vector.tensor_tensor(b2[:, blo:bhi], a2[:, blo:bhi],
                                a2[:, blo + 2:bhi + 2], ADD)
        b23 = b2[:].rearrange("p (h w) -> p h w", w=VPW)
        vp3 = vp[:].rearrange("p (h w) -> p h w", w=VPW)
        cs3 = cs[:].rearrange("p (h w) -> p h w", w=W)
        nc.vector.tensor_tensor(cs3[:, r0:r1, :], b23[:, r0:r1, 0:W],
                                vp3[:, r0:r1, BLK - 1:VPW], ADD)
        # block_mask = (C == 0): threshold in-place at 4x
        nc.vector.tensor_scalar(cs3[:, r0:r1, :], cs3[:, r0:r1, :], 0.5,
                                None, IS_LT)
        return cs

    def emit_mask(t, col, rows=None):
        """mask -> fp8 store + fused count accum on ACT."""
        r0, r1 = (0, H) if rows is None else rows
        cs = state[t]["cs"]
        msl = mask_store[:].rearrange("p (t h w) -> p t h w", t=NTILES, w=W)
        cs3 = cs[:].rearrange("p (h w) -> p h w", w=W)
        nc.scalar.activation(msl[:, t, r0:r1, :], cs3[:, r0:r1, :], IDENT,
                             accum_out=partials[:, col:col + 1])

    RB = 10.0  # pseudo-ms per tile: scheduler ordering only
    for t in range(NTILES):
        with tc.tile_wait_until(t * RB):
            emit_front(t)
        if t >= 1 and t - 1 < NTILES - 1:
            with tc.tile_wait_until(t * RB + 2.5):
                emit_h(t - 1)
            with tc.tile_wait_until(t * RB + 3.0):
                emit_mask(t - 1, t - 1)

    # tile 7 in row-thirds to shorten the critical path into the count
    R3 = [(0, 19), (19, 38), (38, H)]
    with tc.tile_wait_until(8 * RB):
        phead = None
        for i, (r0, r1) in enumerate(R3):
            emit_h(7, rows=(r0, r1))
            emit_mask(7, 7 + i, rows=(r0, r1))
            if i == 1:
                phead = stats_pool.tile([P, 1], F32, name=f"phead{rep}",
                                        tag="phead")
                nc.vector.tensor_reduce(phead[:], partials[:, 0:9],
                                        mybir.AxisListType.X, ADD)
        ptot = stats_pool.tile([P, 1], F32, name=f"ptot{rep}", tag="ptot")
        nc.vector.tensor_tensor(ptot[:], phead[:], partials[:, 9:10], ADD)

        # ---------- global count -> scale = M / count_ones ----------
        from concourse import bass_isa
        pall = stats_pool.tile([P, 1], F32, name=f"pall{rep}", tag="pall")
        nc.gpsimd.partition_all_reduce(pall[:], ptot[:], channels=P,
                                       reduce_op=bass_isa.ReduceOp.add)
        if single_core:
            tot_sb = pall
        else:
            cc_in = dram_pool.tile([P, 1], F32, name=f"cc_in{rep}",
                                   tag="cc_in")
            cc_out = dram_pool.tile([P, 1], F32, name=f"cc_out{rep}",
                                    tag="cc_out")
            nc.scalar.dma_start(cc_in[:], pall[:])
            nc.gpsimd.collective_compute(
                "AllReduce", ADD,
                replica_groups=[list(range(N_CORES))],
                ins=[cc_in.opt()], outs=[cc_out.opt()])
            tot_sb = stats_pool.tile([P, 1], F32, name=f"tot{rep}",
                                     tag="tot")
            nc.scalar.dma_start(tot_sb[:], cc_out[:])
        # scale = M / count: fold 1/M into the reciprocal input
        rin = stats_pool.tile([P, 1], F32, name=f"rin{rep}", tag="rin")
        nc.vector.tensor_scalar_mul(rin[:], tot_sb[:], 1.0 / COUNT_M)
        scale_sb = stats_pool.tile([P, 1], F32, name=f"scale{rep}",
                                   tag="scale")
        nc.vector.reciprocal(scale_sb[:], rin[:])

    # ---------------- phase 2: out = (x*scale)*mask ----------------
    # halves: the first store launches after half a multiply; stores
    # alternate the two HWDGE queues
    for t in range(NTILES):
        xt = xts[t]
        qeng = nc.sync if t % 2 == 0 else nc.scalar
        # tile 0 in quarters so the first store launches sooner (the DMA
        # engines are idle between the last load and it); per-pair ts so
        # the scheduler doesn't coalesce DVE sem updates across pairs
        nparts = 4 if t == 0 else 2
        PX = OPIX // nparts
        for h in range(nparts):
            with tc.tile_wait_until(9 * RB + t + 0.2 * h):
                sl = slice(h * PX, (h + 1) * PX)
                nc.vector.scalar_tensor_tensor(
                    xt[:, sl], xt[:, sl], scale_sb[:, 0:1],
                    mask_store[:,
                               t * OPIX + h * PX:t * OPIX + (h + 1) * PX],
                    MULT, MULT)
                qeng.dma_start(out_ap[bass.ts(t, P), sl], xt[:, sl])


def _get_nc():
    if "nc" not in _CACHE:
        _CACHE["nc"] = _build()
    return _CACHE["nc"]


def kernel(x: np.ndarray, noise: np.ndarray) -> np.ndarray:
    x = np.asarray(x, dtype=np.float32)
    noise = np.asarray(noise, dtype=np.float32)
    assert x.shape == (B, C, H, W) and noise.shape == (B, C, NH, NW)
    nc = _get_nc()
    in_maps = []
    for i in range(N_CORES):
        xs = np.ascontiguousarray(x[i * B_SH:(i + 1) * B_SH]).reshape(
            IMGS, OPIX)
        ns = np.ascontiguousarray(noise[i * B_SH:(i + 1) * B_SH]).reshape(
            IMGS, NPIX)
        in_maps.append({"x": xs, "noise": ns})
    res = run_bass_kernel_spmd(nc, in_maps, list(range(N_CORES)))
    out = np.empty((B, C, H, W), dtype=np.float32)
    for i in range(N_CORES):
        out[i * B_SH:(i + 1) * B_SH] = res.results[i]["out"].reshape(
            B_SH, C, H, W)
    return out


# revision 43
# speedup vs baseline: 1.0488x; 1.0122x over previous
"""DropBlock kernel for Trainium2, 8 NeuronCores, batch-sharded data parallel.

Reference computation (B,C,H,W = 128,64,56,56, block=5, gamma=0.02):
    drop    = (noise < gamma)                       # (B,C,52,52) corner drops
    dilated = maxpool5x5_full_pad(drop)             # (B,C,56,56)
    block_mask = 1 - dilated
    out = block_mask * x * (numel / sum(block_mask))

Kernel formulation (exact -- all intermediates are small integers):
    m = (noise < gamma) in {0,1}                    # Pool tensor_scalar
    C = conv5x5_fullpad(m)   (separable: vertical then horizontal box sum)
    block_mask = (C < 0.5)   == (C == 0) == not dilated
    count = sum(block_mask) via fused ACT accum; AllReduce across 8 cores.

Engine assignment (each phase-1 stage on its own engine, pipelined by
tile; the Pool engine only supports TensorScalar/memset/ISA ops and the
PE only contracts over partitions -- both constraints shaped this):
    Pool: m = tensor_scalar(noise, is_lt gamma)  (the only legal Pool op)
    PE  : vertical 5-row box sum as 5 identity-matmuls with row-shifted
          moving APs accumulated into PSUM (416-col row-aligned chunks;
          contraction over partitions is a no-op via the identity)
    ACT : PSUM -> SBUF bf16 drain into the 60-wide zero-padded layout,
          and the final mask write (fp8) + fused count accumulation
    DVE : horizontal box sum (3 log-step adds), threshold (tensor_scalar
          is_lt at 4x), and the phase-2 (x*scale)*mask multiplies
Tile 7's horizontal chain runs in row-thirds to shorten the critical
path into the count -> AllReduce -> scale; stores start right after.
"""

import sys

sys.path.insert(0, "/opt/trn_rl_repo")

import numpy as np

import concourse.bacc as bacc
import concourse.bass as bass
import concourse.tile as tile
import concourse.mybir as mybir
from concourse.masks import make_identity
from concourse.bass_utils import run_bass_kernel_spmd

N_CORES = 8
B, C, H, W = 128, 64, 56, 56
BLK = 5
GAMMA = 0.02
NH, NW = H - (BLK - 1), W - (BLK - 1)  # 52, 52 noise dims
B_SH = B // N_CORES  # 16 batches per core
IMGS = B_SH * C  # 1024 images per core
P = 128  # partitions
NTILES = IMGS // P  # 8 tiles per core
NPIX = NH * NW  # 2704 noise pixels/image
OPIX = H * W  # 3136 out pixels/image
TROWS = NH + 2 * (BLK - 1)  # 60 rows in zero-padded indicator buffer
TFLAT = TROWS * NW  # 3120
VPW = NW + 2 * (BLK - 1)  # 60 cols in zero-padded horizontal buffer
HV = H * VPW  # 3360
COUNT_M = float(B * C * H * W)  # 25690112.0

F32 = mybir.dt.float32
BF16 = mybir.dt.bfloat16
FP8 = mybir.dt.float8e4
MULT = mybir.AluOpType.mult
ADD = mybir.AluOpType.add
IS_LT = mybir.AluOpType.is_lt
IDENT = mybir.ActivationFunctionType.Identity

CHROWS = 8  # V rows per PE/PSUM chunk (416 cols = 1 PSUM bank)
NCHUNK = H // CHROWS  # 7 chunks per tile
CHW = CHROWS * NW  # 416

_CACHE = {}


def _build(single_core=False, repeat=1, no_cc=False):
    """Build + compile the SPMD bass module once.

    single_core=True builds a collective-free variant (per-core count used
    as the global count) for cost-model simulation only.  repeat>1 unrolls
    the pipeline k times (benchmarking only).  no_cc=True skips the
    AllReduce on the 8-core build (timing probe only).
    """
    nc = bacc.Bacc("TRN2", target_bir_lowering=False, debug=False,
                   num_devices=1 if single_core else N_CORES)
    noise_ap = nc.dram_tensor("noise", [IMGS, NPIX], F32,
                              kind="ExternalInput").ap()
    x_ap = nc.dram_tensor("x", [IMGS, OPIX], F32, kind="ExternalInput").ap()
    out_ap = nc.dram_tensor("out", [IMGS, OPIX], F32,
                            kind="ExternalOutput").ap()

    with tile.TileContext(nc) as tc:
        with (
            tc.tile_pool(name="nraw", bufs=4) as nraw_pool,
            tc.tile_pool(name="work", bufs=1) as work_pool,
            tc.tile_pool(name="vp", bufs=2) as vp_pool,
            tc.tile_pool(name="dmin", bufs=2) as dmin_pool,
            tc.tile_pool(name="mask", bufs=1) as mask_pool,
            tc.tile_pool(name="stats", bufs=1) as stats_pool,
            tc.tile_pool(name="xio", bufs=8) as x_pool,
            tc.tile_pool(name="psum", bufs=8, space="PSUM") as psum_pool,
            tc.tile_pool(name="dram", bufs=1, space="DRAM") as dram_pool,
        ):
            mask_store = mask_pool.tile([P, NTILES * OPIX], FP8)
            # count columns: tiles 0..6 -> 0..6, tile-7 thirds -> 7..9
            partials = stats_pool.tile([P, 10], F32)
            ident = stats_pool.tile([P, P], BF16)
            make_identity(nc, ident[:])
            # warm the ACT Identity table during the DMA lead-in
            warm = stats_pool.tile([P, 1], F32)
            warm2 = stats_pool.tile([P, 1], F32)
            nc.scalar.activation(warm[:], warm[:], IDENT,
                                 accum_out=warm2[:, 0:1])
            # warm the PE p-state: ~3us of dummy matmuls during the DMA
            # lead-in so real conv matmuls start at full clock
            wps = psum_pool.tile([P, P], F32, name="warmps", tag="warmps",
                                 bufs=1)
            for _ in range(18):
                nc.tensor.matmul(wps[:], ident[:], ident[:], start=True,
                                 stop=True)

            pools = dict(nraw=nraw_pool, work=work_pool, vp=vp_pool,
                         dmin=dmin_pool, x=x_pool, psum=psum_pool,
                         stats=stats_pool, dram=dram_pool)
            for rep in range(repeat):
                _emit_once(nc, tc, noise_ap, x_ap, out_ap, mask_store,
                           partials, ident, pools, single_core or no_cc,
                           rep)

    nc.compile()
    return nc


def _emit_once(nc, tc, noise_ap, x_ap, out_ap, mask_store, partials, ident,
               pools, single_core, rep):
    nraw_pool = pools["nraw"]
    work_pool = pools["work"]
    vp_pool = pools["vp"]
    dmin_pool = pools["dmin"]
    x_pool = pools["x"]
    stats_pool = pools["stats"]
    dram_pool = pools["dram"]
    psum_pool = pools["psum"]

    xts = {}
    state = {}
    pre = {}
    # pre-allocate the pad-carrying buffers and memset the ZERO pads once,
    # up-front (in-loop memsets would queue behind real Pool/DVE work)
    pre["mb"] = [work_pool.tile([P, TFLAT], BF16, name=f"mbp{rep}_{i}",
                                tag="mb", bufs=3) for i in range(3)]
    pre["vp"] = [vp_pool.tile([P, HV], BF16, name=f"vpp{rep}_{i}",
                              tag="vp") for i in range(2)]
    if rep == 0:
        for mbp in pre["mb"]:
            nc.gpsimd.memset(mbp[:, 0:(BLK - 1) * NW], 0.0)
            nc.gpsimd.memset(mbp[:, (NH + BLK - 1) * NW:TFLAT], 0.0)
        for vpp in pre["vp"]:
            v3 = vpp[:].rearrange("p (h w) -> p h w", w=VPW)
            nc.gpsimd.memset(v3[:, :, 0:BLK - 1], 0.0)
            nc.gpsimd.memset(v3[:, :, W:VPW], 0.0)

    def emit_front(t):
        """noise loads, x prefetch, Pool indicator, PE conv, ACT drain."""
        # two half-tiles in the nh ring; tile 0 issues quarter-DMAs and
        # quarter indicator ops into the same buffers so the pipeline
        # fill chases at 13-row granularity
        nhs = []
        for hf in range(2):
            nh = nraw_pool.tile([P, NPIX // 2], F32,
                                name=f"nh{rep}_{t}_{hf}", tag="nh")
            nhs.append(nh)
            base = hf * (NPIX // 2)
            if t == 0:
                nc.sync.dma_start(nh[:, 0:NPIX // 4],
                                  noise_ap[bass.ts(t, P),
                                           base:base + NPIX // 4])
                nc.sync.dma_start(nh[:, NPIX // 4:NPIX // 2],
                                  noise_ap[bass.ts(t, P),
                                           base + NPIX // 4:base + NPIX // 2])
            else:
                nc.sync.dma_start(nh[:],
                                  noise_ap[bass.ts(t, P),
                                           base:base + NPIX // 2])
        # x loads ride the other HWDGE queue (ACT's), in halves
        xts[t] = x_pool.tile([P, OPIX], F32, name=f"xt{rep}_{t}", tag="xt")
        with tc.tile_wait_until(t * 10.0 + 5.0):
            nc.scalar.dma_start(xts[t][:, 0:OPIX // 2],
                                x_ap[bass.ts(t, P), 0:OPIX // 2])
            nc.scalar.dma_start(xts[t][:, OPIX // 2:OPIX],
                                x_ap[bass.ts(t, P), OPIX // 2:OPIX])

        # drop indicator m in {0,1}, rows 4..55 of the 60-row zero-padded
        # buffer; exact f32 compare against gamma, bf16 out (Pool engine)
        mb = pre["mb"][t] if t < 3 else work_pool.tile(
            [P, TFLAT], BF16, name=f"mb{rep}_{t}", tag="mb", bufs=3)
        nparts = 4 if t == 0 else 2
        rows_per = NH // nparts  # 13 or 26 noise rows per part
        for q in range(nparts):
            r0 = (BLK - 1) + q * rows_per
            nh = nhs[q // 2] if nparts == 4 else nhs[q]
            lo = (q % 2) * (NPIX // 4) if nparts == 4 else 0
            nc.gpsimd.tensor_scalar(
                mb[:, r0 * NW:(r0 + rows_per) * NW],
                nh[:, lo:lo + rows_per * NW], GAMMA, None, IS_LT)

        # vertical 5-row box sum on PE: V[i] = sum_j m[i + 52j], computed
        # as 5 identity-matmuls with row-shifted moving APs accumulated in
        # PSUM, in 8-row chunks (416 cols = 1 bank); ACT drains each chunk
        # into the padded horizontal buffer as bf16 (values 0..5, exact)
        vp = pre["vp"][t] if t < 2 else vp_pool.tile(
            [P, HV], BF16, name=f"vp{rep}_{t}", tag="vp")
        vp3 = vp[:].rearrange("p (h w) -> p h w", w=VPW)
        for c in range(NCHUNK):
            pt = psum_pool.tile([P, CHW], F32, name=f"ps{rep}_{t}_{c}",
                                tag="ps", bufs=7)
            for j in range(BLK):
                nc.tensor.matmul(
                    pt[:], ident[:],
                    mb[:, c * CHW + NW * j:c * CHW + NW * j + CHW],
                    start=(j == 0), stop=(j == BLK - 1))
            pt3 = pt[:].rearrange("p (h w) -> p h w", w=NW)
            dst = vp3[:, c * CHROWS:(c + 1) * CHROWS, BLK - 1:BLK - 1 + NW]
            if c >= 5 and t < NTILES - 2:
                # last two chunks drain on DVE (scheduled after the
                # previous tile's h-chain) to unload the ACT stream;
                # tiles 6/7 keep ACT drains (hinted ahead of the t-1
                # mask) so the DVE end-game chain starts sooner
                with tc.tile_wait_until(t * 10.0 + 2.9):
                    nc.vector.tensor_copy(dst, pt3[:, :, :])
            elif t >= NTILES - 2:
                with tc.tile_wait_until((t - 1) * 10.0 + 2.8):
                    nc.scalar.activation(dst, pt3[:, :, :], IDENT)
            else:
                nc.scalar.activation(dst, pt3[:, :, :], IDENT)
        state[t] = {"vp": vp}
        return vp

    def emit_h(t, rows=None):
        """horizontal box sum + threshold on DVE for row range [r0, r1)."""
        r0, r1 = (0, H) if rows is None else rows
        vp = state[t]["vp"]
        a2 = state[t].get("a2")
        if a2 is None:
            a2 = work_pool.tile([P, HV], BF16, name=f"a2{rep}_{t}",
                                tag="a2", bufs=2)
            state[t]["a2"] = a2
            b2 = work_pool.tile([P, HV], BF16, name=f"b2{rep}_{t}",
                                tag="b2", bufs=1)
            state[t]["b2"] = b2
            cs = dmin_pool.tile([P, OPIX], BF16, name=f"cs{rep}_{t}",
                                tag="cs")
            state[t]["cs"] = cs
        b2 = state[t]["b2"]
        cs = state[t]["cs"]
        alo, ahi = r0 * VPW, min((r1 - 1) * VPW + W + 2, HV - 1)
        nc.vector.tensor_tensor(a2[:, alo:ahi], vp[:, alo:ahi],
                                vp[:, alo + 1:ahi + 1], ADD)
        blo, bhi = r0 * VPW, min((r1 - 1) * VPW + W, HV - 2)
        nc.vector.tensor_tensor(b2[:, blo:bhi], a2[:, blo:bhi],
                                a2[:, blo + 2:bhi + 2], ADD)
        b23 = b2[:].rearrange("p (h w) -> p h w", w=VPW)
        vp3 = vp[:].rearrange("p (h w) -> p h w", w=VPW)
        cs3 = cs[:].rearrange("p (h w) -> p h w", w=W)
        nc.vector.tensor_tensor(cs3[:, r0:r1, :], b23[:, r0:r1, 0:W],
                                vp3[:, r0:r1, BLK - 1:VPW], ADD)
        # block_mask = (C == 0): threshold in-place at 4x
        nc.vector.tensor_scalar(cs3[:, r0:r1, :], cs3[:, r0:r1, :], 0.5,
                                None, IS_LT)
        return cs

    def emit_mask(t, col, rows=None):
        """mask -> fp8 store + fused count accum on ACT."""
        r0, r1 = (0, H) if rows is None else rows
        cs = state[t]["cs"]
        msl = mask_store[:].rearrange("p (t h w) -> p t h w", t=NTILES, w=W)
        cs3 = cs[:].rearrange("p (h w) -> p h w", w=W)
        nc.scalar.activation(msl[:, t, r0:r1, :], cs3[:, r0:r1, :], IDENT,
                             accum_out=partials[:, col:col + 1])

    RB = 10.0  # pseudo-ms per tile: scheduler ordering only
    for t in range(NTILES):
        with tc.tile_wait_until(t * RB):
            emit_front(t)
        if t >= 1 and t - 1 < NTILES - 1:
            with tc.tile_wait_until(t * RB + 2.5):
                emit_h(t - 1)
            with tc.tile_wait_until(t * RB + 3.0):
                emit_mask(t - 1, t - 1)

    # tile 7 in row-thirds to shorten the critical path into the count
    R3 = [(0, 19), (19, 38), (38, H)]
    with tc.tile_wait_until(8 * RB):
        phead = None
        for i, (r0, r1) in enumerate(R3):
            emit_h(7, rows=(r0, r1))
            emit_mask(7, 7 + i, rows=(r0, r1))
            if i == 1:
                phead = stats_pool.tile([P, 1], F32, name=f"phead{rep}",
                                        tag="phead")
                nc.vector.tensor_reduce(phead[:], partials[:, 0:9],
                                        mybir.AxisListType.X, ADD)
        ptot = stats_pool.tile([P, 1], F32, name=f"ptot{rep}", tag="ptot")
        nc.vector.tensor_tensor(ptot[:], phead[:], partials[:, 9:10], ADD)

        # ---------- global count -> scale = M / count_ones ----------
        from concourse import bass_isa
        pall = stats_pool.tile([P, 1], F32, name=f"pall{rep}", tag="pall")
        nc.gpsimd.partition_all_reduce(pall[:], ptot[:], channels=P,
                                       reduce_op=bass_isa.ReduceOp.add)
        if single_core:
            tot_sb = pall
        else:
            cc_in = dram_pool.tile([P, 1], F32, name=f"cc_in{rep}",
                                   tag="cc_in")
            cc_out = dram_pool.tile([P, 1], F32, name=f"cc_out{rep}",
                                    tag="cc_out")
            nc.scalar.dma_start(cc_in[:], pall[:])
            nc.gpsimd.collective_compute(
                "AllReduce", ADD,
                replica_groups=[list(range(N_CORES))],
                ins=[cc_in.opt()], outs=[cc_out.opt()])
            tot_sb = stats_pool.tile([P, 1], F32, name=f"tot{rep}",
                                     tag="tot")
            nc.scalar.dma_start(tot_sb[:], cc_out[:])
        # scale = M / count: fold 1/M into the reciprocal input
        rin = stats_pool.tile([P, 1], F32, name=f"rin{rep}", tag="rin")
        nc.vector.tensor_scalar_mul(rin[:], tot_sb[:], 1.0 / COUNT_M)
        scale_sb = stats_pool.tile([P, 1], F32, name=f"scale{rep}",
                                   tag="scale")
        nc.vector.reciprocal(scale_sb[:], rin[:])

    # ---------------- phase 2: out = (x*scale)*mask ----------------
    # halves: the first store launches after half a multiply; stores
    # alternate the two HWDGE queues
    for t in range(NTILES):
        xt = xts[t]
        qeng = nc.sync if t % 2 == 0 else nc.scalar
        # tile 0 in quarters so the first store launches sooner (the DMA
        # engines are idle between the last load and it); per-pair ts so
        # the scheduler doesn't coalesce DVE sem updates across pairs
        nparts = 4 if t == 0 else 2
        PX = OPIX // nparts
        for h in range(nparts):
            with tc.tile_wait_until(9 * RB + t + 0.2 * h):
                sl = slice(h * PX, (h + 1) * PX)
                nc.vector.scalar_tensor_tensor(
                    xt[:, sl], xt[:, sl], scale_sb[:, 0:1],
                    mask_store[:,
                               t * OPIX + h * PX:t * OPIX + (h + 1) * PX],
                    MULT, MULT)
                qeng.dma_start(out_ap[bass.ts(t, P), sl], xt[:, sl])


def _get_nc():
    if "nc" not in _CACHE:
        _CACHE["nc"] = _build()
    return _CACHE["nc"]


def kernel(x: np.ndarray, noise: np.ndarray) -> np.ndarray:
    x = np.asarray(x, dtype=np.float32)
    noise = np.asarray(noise, dtype=np.float32)
    assert x.shape == (B, C, H, W) and noise.shape == (B, C, NH, NW)
    nc = _get_nc()
    in_maps = []
    for i in range(N_CORES):
        xs = np.ascontiguousarray(x[i * B_SH:(i + 1) * B_SH]).reshape(
            IMGS, OPIX)
        ns = np.ascontiguousarray(noise[i * B_SH:(i + 1) * B_SH]).reshape(
            IMGS, NPIX)
        in_maps.append({"x": xs, "noise": ns})
    res = run_bass_kernel_spmd(nc, in_maps, list(range(N_CORES)))
    out = np.empty((B, C, H, W), dtype=np.float32)
    for i in range(N_CORES):
        out[i * B_SH:(i + 1) * B_SH] = res.results[i]["out"].reshape(
            B_SH, C, H, W)
    return out
